# revision 6
# baseline (speedup 1.0000x reference)
"""Trainium2 Bass kernel for nn_DepthAwareFE.

Strategy (8 cores, SPMD):
  Shard: (sample n, vertical half h) -> core 2n+h. Odd cores process a
  vertically FLIPPED copy of the sample (with row-flipped 3x3 conv weights),
  so one program serves both halves; host flips outputs back.
  Each core computes, for its half (in local "top" coordinates):
    - bilinear 2x upsample rows 0..130 (row interp per-row on DVE, column
      interp via even/odd affine two-tap form on DVE)
    - conv3x3 256->128 (+folded BN +ReLU) as 9x2 shifted matmuls -> d rows 0..129
    - conv1x1 128->96 (+bias) -> depth rows 0..129 (rows 0..127 are output)
    - 2x2 avg-pool (folded 0.25 into grouped conv weights) -> dg rows 0..64
    - grouped conv 96->12 (block-diag matmuls) -> depth_guide rows 0..63
    - feat = relu(bn(conv1x1 acf1(x))) computed transposed (pixel-major)
    - partial energy (12,256) = q @ feat^T over local pixels
    - pair AllReduce of energy -> softmax(min-trick) -> attn
    - fold attn into acf2: G^T = attn @ w_acf2^T; final = G@q + b + x
Weights/BN are folded on the host; all matmuls run fp32 (or fp32r).
"""
import os
import sys

sys.path.insert(0, '/opt/trn_rl_repo')

import numpy as np

BN_EPS = 1e-5
H = W = 128
C = 256
UW = 256
RD = 130          # d/depth rows per core (2 extra for pool halo)
UPR = RD + 1      # upsample rows computed per core (plus implicit top zero pad)
XR = 66           # x rows shipped per core for the upsample
CH = 6            # d rows per conv chunk
N_CORES = 8
USE_F32R = os.environ.get("KERNEL_F32R", "1") == "1"

_cache = {}


def _tables():
    # mirror reference f32 arithmetic exactly (align_corners=True, 128->256)
    ys = np.arange(2 * H, dtype=np.float32) * np.float32((H - 1) / (2 * H - 1))
    y0 = np.floor(ys).astype(np.int64)
    wy = (ys - y0.astype(np.float32)).astype(np.float32)
    row_y0 = y0[:UPR]
    row_wy = wy[:UPR]
    xs = np.arange(2 * W, dtype=np.float32) * np.float32((W - 1) / (2 * W - 1))
    x0 = np.floor(xs).astype(np.int64)
    wx = (xs - x0.astype(np.float32)).astype(np.float32)
    Ae = np.zeros(W, np.float32); Be = np.zeros(W, np.float32)
    Ao = np.zeros(W, np.float32); Bo = np.zeros(W, np.float32)
    for k in range(W):
        for (parity, lo, A, B) in ((0, k - 1, Ae, Be), (1, k, Ao, Bo)):
            xp = 2 * k + parity
            t0, t1 = int(x0[xp]), min(int(x0[xp]) + 1, W - 1)
            cmap = {t0: 1.0 - float(wx[xp])}
            cmap[t1] = cmap.get(t1, 0.0) + float(wx[xp])
            A[k] = cmap.get(lo, 0.0)
            B[k] = cmap.get(lo + 1, 0.0)
    return row_y0, row_wy, Ae, Be, Ao, Bo


def _chunks():
    """Conv row chunking: list of (r0, nrows, up_lo, up_hi, x_lo, x_hi)."""
    row_y0, _, _, _, _, _ = _tables()
    out = []
    r0 = 0
    while r0 < RD:
        nr = min(CH, RD - r0)
        up_lo = max(0, r0 - 1)
        up_hi = r0 + nr          # inclusive
        x_lo = int(row_y0[up_lo])
        x_hi = int(row_y0[up_hi]) + 1
        out.append((r0, nr, up_lo, up_hi, x_lo, x_hi))
        r0 += nr
    return out


def _build_nc(use_f32r):
    import concourse.bass as bass
    import concourse.bacc as bacc
    import concourse.mybir as mybir
    import concourse.tile as tile
    from concourse.alu_op_type import AluOpType
    from concourse.bass import broadcast_tensor_aps

    f32 = mybir.dt.float32
    f32r = mybir.dt.float32r
    AF = mybir.ActivationFunctionType
    P = 128

    rdt = f32r if use_f32r else f32

    row_y0, row_wy, _, _, _, _ = _tables()
    chunks = _chunks()

    nc = bacc.Bacc("TRN2", target_bir_lowering=False, debug=False, num_devices=N_CORES)

    xin = nc.dram_tensor("xin", [P, 2, XR, W], f32, kind="ExternalInput").ap()
    w1t = nc.dram_tensor("w1t", [P, 2, 9, P], f32, kind="ExternalInput").ap()
    b1d = nc.dram_tensor("b1d", [P, 1], f32, kind="ExternalInput").ap()
    w2t = nc.dram_tensor("w2t", [P, 96], f32, kind="ExternalInput").ap()
    b2d = nc.dram_tensor("b2d", [96, 1], f32, kind="ExternalInput").ap()
    wddt = nc.dram_tensor("wddt", [96, 9, 12], f32, kind="ExternalInput").ap()
    bddd = nc.dram_tensor("bddd", [12, 1], f32, kind="ExternalInput").ap()
    wa1t = nc.dram_tensor("wa1t", [P, 2, 256], f32, kind="ExternalInput").ap()
    b2row = nc.dram_tensor("b2row", [1, 256], f32, kind="ExternalInput").ap()
    wa2t = nc.dram_tensor("wa2t", [P, 2, 256], f32, kind="ExternalInput").ap()
    b2at = nc.dram_tensor("b2at", [P, 2], f32, kind="ExternalInput").ap()
    colw = nc.dram_tensor("colw", [1, 4 * W], f32, kind="ExternalInput").ap()
    identd = nc.dram_tensor("identd", [16, 16], f32, kind="ExternalInput").ap()

    depth_o = nc.dram_tensor("depth_o", [96, 2 * 64, UW], f32, kind="ExternalOutput").ap()
    dg_o = nc.dram_tensor("dg_o", [12, 64, W], f32, kind="ExternalOutput").ap()
    fin_o = nc.dram_tensor("fin_o", [P, 2, 64, W], f32, kind="ExternalOutput").ap()

    with tile.TileContext(nc) as tc:
        from contextlib import ExitStack
        with ExitStack() as ctx:
            const = ctx.enter_context(tc.tile_pool(name="const", bufs=1))
            dramp = ctx.enter_context(tc.tile_pool(name="dramp", bufs=1, space="DRAM"))

            # ---- load constants ----
            w1 = const.tile([P, 2, 9, P], f32)
            nc.sync.dma_start(w1[:], w1t[:])
            b1 = const.tile([P, 1], f32)
            nc.sync.dma_start(b1[:], b1d[:])
            w2 = const.tile([P, 96], f32)
            nc.sync.dma_start(w2[:], w2t[:])
            b2 = const.tile([96, 1], f32)
            nc.sync.dma_start(b2[:], b2d[:])
            wdd = const.tile([96, 9, 12], f32)
            nc.sync.dma_start(wdd[:], wddt[:])
            bdd = const.tile([12, 1], f32)
            nc.sync.dma_start(bdd[:], bddd[:])
            wa1 = const.tile([P, 2, 256], f32)
            nc.sync.dma_start(wa1[:], wa1t[:])
            wa2 = const.tile([P, 2, 256], f32)
            nc.sync.dma_start(wa2[:], wa2t[:])
            b2a = const.tile([P, 2], f32)
            nc.sync.dma_start(b2a[:], b2at[:])
            ident = const.tile([16, 16], f32)
            nc.sync.dma_start(ident[:], identd[:])
            b2row_sb = const.tile([1, 256], f32)
            nc.sync.dma_start(b2row_sb[:], b2row[:])
            b2rep = const.tile([P, 256], f32)
            nc.gpsimd.partition_broadcast(b2rep[:], b2row_sb[:])
            colw_sb = const.tile([1, 4 * W], f32)
            nc.sync.dma_start(colw_sb[:], colw[:])
            cw = const.tile([P, 4, W], f32)
            nc.gpsimd.partition_broadcast(cw[:].rearrange("p a b -> p (a b)"), colw_sb[:])

            w1r = const.tile([P, 2, 9, P], rdt)
            nc.vector.tensor_copy(w1r[:], w1[:])
            w2r = const.tile([P, 96], rdt)
            nc.vector.tensor_copy(w2r[:], w2[:])
            wddr = const.tile([96, 9, 12], rdt)
            nc.vector.tensor_copy(wddr[:], wdd[:])

            # dg accumulation buffer: row 0 = zero pad, rows 1..65 = dg rows 0..64;
            # cols 0 and 129 zero pad
            dgbuf = const.tile([96, 66, 130], rdt)
            nc.any.memset(dgbuf[:, 0, :].bitcast(f32), 0.0)
            nc.any.memset(dgbuf[:, :, 0:130:129].bitcast(f32), 0.0)

            qT = const.tile([P, 64, 12], rdt)     # q transposed, per 128-px tile
            dgd = dramp.tile([12, 64, W], f32)    # dg copy in DRAM (tracked RAW)
            edram = dramp.tile([12, 256], f32)
            erdram = dramp.tile([12, 256], f32)

            # ---- phase A: upsample + conv3x3 + conv1x1 + pool ----
            with ExitStack() as actx:
                xpool = actx.enter_context(tc.tile_pool(name="xpool", bufs=2))
                dpool = actx.enter_context(tc.tile_pool(name="dpool", bufs=2))
                urpool = actx.enter_context(tc.tile_pool(name="urpool", bufs=2))
                uppool = actx.enter_context(tc.tile_pool(name="uppool", bufs=2))
                tpool = actx.enter_context(tc.tile_pool(name="tpool", bufs=2))
                sbA = actx.enter_context(tc.tile_pool(name="sbA", bufs=3))
                psA = actx.enter_context(tc.tile_pool(name="psA", bufs=2, space="PSUM"))
                psD = actx.enter_context(tc.tile_pool(name="psD", bufs=2, space="PSUM"))

                for (r0, nr, up_lo, up_hi, x_lo, x_hi) in chunks:
                    nx = x_hi - x_lo + 1
                    nup = up_hi - up_lo + 1          # real up rows
                    nbuf = nr + 2                     # buffer rows incl pad/halo
                    xt = xpool.tile([P, 2, 6, W], f32, name="xt")
                    nc.sync.dma_start(xt[:, :, :nx, :], xin[:, :, x_lo:x_hi + 1, :])
                    dif = dpool.tile([P, 2, 5, W], f32, name="dif")
                    nc.vector.tensor_tensor(
                        dif[:, :, :nx - 1, :], xt[:, :, 1:nx, :], xt[:, :, :nx - 1, :],
                        AluOpType.subtract)
                    # row interp into padded ur buffer (col 0 and col 129 zero)
                    ur = urpool.tile([P, 2, CH + 2, W + 2], f32, name="ur")
                    nc.any.memset(ur[:, :, :nbuf, 0:W + 2:W + 1], 0.0)
                    pad0 = 1 if r0 == 0 else 0
                    if pad0:
                        nc.any.memset(ur[:, :, 0, :], 0.0)
                    for j in range(nup):
                        y = up_lo + j
                        yl = int(row_y0[y]) - x_lo
                        nc.vector.scalar_tensor_tensor(
                            ur[:, :, pad0 + j, 1:W + 1], dif[:, :, yl, :],
                            float(row_wy[y]), xt[:, :, yl, :],
                            AluOpType.mult, AluOpType.add)
                    # column interp into padded up buffer (cols 0,257 zero)
                    up = uppool.tile([P, 2, CH + 2, 2 * W + 2], rdt, name="up")
                    nc.any.memset(up[:, :, :nbuf, 0:2 * W + 2:2 * W + 1].bitcast(f32), 0.0)
                    tmp = tpool.tile([P, 2, CH + 2, W], f32, name="tmp")
                    for (par, aidx, bidx, off) in ((0, 0, 1, 0), (1, 2, 3, 1)):
                        # out col (2k+par) -> buffer col 1+2k+par
                        src_a = ur[:, :, :nbuf, off:off + W]
                        src_b = ur[:, :, :nbuf, off + 1:off + 1 + W]
                        dst = up[:, :, :nbuf, 1 + par:1 + par + 2 * W:2]
                        wa = cw[:, aidx, :].rearrange("p (a b c) -> p a b c", a=1, b=1)
                        wb = cw[:, bidx, :].rearrange("p (a b c) -> p a b c", a=1, b=1)
                        s1, w1b = broadcast_tensor_aps(src_a, wa)
                        nc.vector.tensor_tensor(dst, s1, w1b, AluOpType.mult)
                        s2, w2b = broadcast_tensor_aps(src_b, wb)
                        t_ = tmp[:, :, :nbuf, :]
                        nc.vector.tensor_tensor(t_, s2, w2b, AluOpType.mult)
                        nc.vector.tensor_tensor(dst, dst, t_, AluOpType.add)
                    # conv3x3 + bias/relu + conv1x1 + bias per row pair
                    for pi in range(nr // 2):
                        gy = r0 + 2 * pi              # global (local-core) d row
                        jb = 2 * pi + (1 - pad0) * 0  # buffer row of out row gy is
                        # buffer row index of up row (gy-1+dy):
                        #   (gy-1+dy) - (up_lo) + pad0 = 2*pi + dy (both pad0 cases)
                        cp = psA.tile([P, 2, UW], f32, name="cp")
                        first = True
                        for dy in range(3):
                            rlo = 2 * pi + dy
                            for g in range(2):
                                for dx in range(3):
                                    nc.tensor.matmul(
                                        cp[:],
                                        w1r[:, g, 3 * dy + dx, :],
                                        up[:, g, rlo:rlo + 2, dx:dx + UW],
                                        start=first, stop=(dy == 2 and g == 1 and dx == 2))
                                    first = False
                        d_sb = sbA.tile([P, 2, UW], rdt, name="d_sb")
                        nc.scalar.activation(d_sb[:], cp[:], AF.Relu, bias=b1[:])
                        dp = psD.tile([96, 2, UW], f32, name="dp")
                        nc.tensor.matmul(dp[:], w2r[:], d_sb[:],
                                         start=True, stop=True)
                        dep = sbA.tile([96, 2, UW], f32, name="dep")
                        nc.scalar.activation(dep[:], dp[:], AF.Identity, bias=b2[:])
                        if gy < 2 * 64:
                            nc.sync.dma_start(depth_o[:, gy:gy + 2, :], dep[:])
                        # pooled sums into dgbuf row (1 + gy//2), cols 1..128
                        pr = 1 + gy // 2
                        t1 = sbA.tile([96, W], f32, name="t1")
                        nc.vector.tensor_tensor(t1[:], dep[:, 0, 0:UW:2],
                                                dep[:, 0, 1:UW:2], AluOpType.add)
                        t2 = sbA.tile([96, W], f32, name="t2")
                        nc.vector.tensor_tensor(t2[:], dep[:, 1, 0:UW:2],
                                                dep[:, 1, 1:UW:2], AluOpType.add)
                        nc.vector.tensor_tensor(dgbuf[:, pr, 1:W + 1], t1[:], t2[:],
                                                AluOpType.add)

            # ---- phase B: grouped conv -> depth_guide (+ transposes for energy) ----
            with ExitStack() as bctx:
                psG = bctx.enter_context(tc.tile_pool(name="psG", bufs=2, space="PSUM"))
                psT = bctx.enter_context(tc.tile_pool(name="psT", bufs=2, space="PSUM"))
                sbB = bctx.enter_context(tc.tile_pool(name="sbB", bufs=3))
                for t in range(16):
                    rt = 4 * t
                    gp = psG.tile([12, 4, W], f32, name="gp")
                    k = 0
                    for dy in range(3):
                        for dx in range(3):
                            nc.tensor.matmul(
                                gp[:], wddr[:, 3 * dy + dx, :],
                                dgbuf[:, rt + dy:rt + dy + 4, dx:dx + W],
                                start=(k == 0), stop=(k == 8))
                            k += 1
                    qsb = sbB.tile([12, 4, W], f32, name="qsb")
                    nc.scalar.activation(qsb[:], gp[:], AF.Identity, bias=bdd[:])
                    nc.sync.dma_start(dg_o[:, rt:rt + 4, :], qsb[:])
                    nc.sync.dma_start(dgd[:, rt:rt + 4, :], qsb[:])
                    for rr in range(4):
                        tp = psT.tile([P, 16], f32, name="tp")
                        nc.tensor.transpose(tp[:12 * 0 + P, :12], qsb[:, rr, :],
                                            ident[:12, :12])
                        nc.scalar.copy(qT[:, rt + rr, :], tp[:, :12])

            # ---- phase C: feat (transposed) + energy + allreduce + attn + G ----
            with ExitStack() as cctx:
                psF = cctx.enter_context(tc.tile_pool(name="psF", bufs=2, space="PSUM"))
                psE = cctx.enter_context(tc.tile_pool(name="psE", bufs=1, space="PSUM"))
                psX = cctx.enter_context(tc.tile_pool(name="psX", bufs=2, space="PSUM"))
                sbC = cctx.enter_context(tc.tile_pool(name="sbC", bufs=3))
                xres = cctx.enter_context(tc.tile_pool(name="xres", bufs=3))
                ep = psE.tile([12, 256], f32, name="ep")
                for i in range(64):
                    xt2 = xres.tile([P, 2, W], f32, name="xt2")
                    nc.sync.dma_start(xt2[:], xin[:, :, i, :])
                    fp = psF.tile([P, 256], f32, name="fp")
                    nc.tensor.matmul(fp[:], xt2[:, 0, :], wa1[:, 0, :],
                                     start=True, stop=False)
                    nc.tensor.matmul(fp[:], xt2[:, 1, :], wa1[:, 1, :],
                                     start=False, stop=True)
                    fsb = sbC.tile([P, 256], rdt, name="fsb")
                    nc.vector.tensor_tensor(fsb[:], fp[:], b2rep[:], AluOpType.add)
                    nc.vector.tensor_scalar_max(fsb[:], fsb[:], 0.0)
                    nc.tensor.matmul(ep[:], qT[:, i, :], fsb[:],
                                     start=(i == 0), stop=(i == 63))
                esb = sbC.tile([12, 256], f32, name="esb")
                nc.vector.tensor_copy(esb[:], ep[:])
                nc.sync.dma_start(edram[:], esb[:])
                nc.gpsimd.collective_compute(
                    "AllReduce", AluOpType.add,
                    replica_groups=[[0, 1], [2, 3], [4, 5], [6, 7]],
                    ins=[edram.opt()], outs=[erdram.opt()])
                er = sbC.tile([12, 256], f32, name="er")
                nc.sync.dma_start(er[:], erdram[:])
                mn = sbC.tile([12, 1], f32, name="mn")
                nc.vector.reduce_max(mn[:], er[:], mybir.AxisListType.X,
                                     op=AluOpType.min)
                ex = sbC.tile([12, 256], f32, name="ex")
                nc.scalar.activation(ex[:], er[:], AF.Exp, bias=mn[:], scale=-1.0)
                sm = sbC.tile([12, 1], f32, name="sm")
                nc.vector.reduce_sum(sm[:], ex[:], mybir.AxisListType.X)
                rc = sbC.tile([12, 1], f32, name="rc")
                nc.vector.reciprocal(rc[:], sm[:])
                at = sbC.tile([12, 256], f32, name="at")
                nc.vector.tensor_scalar_mul(at[:], ex[:], rc[:])
                # attn^T (2x transpose) then G^T = attn @ w_acf2^T
                atT = sbC.tile([P, 2, 12], f32, name="atT")
                for h in range(2):
                    tp2 = psX.tile([P, 16], f32, name="tp2")
                    nc.tensor.transpose(tp2[:, :12], at[:, 128 * h:128 * (h + 1)],
                                        ident[:12, :12])
                    nc.scalar.copy(atT[:, h, :], tp2[:, :12])
                gtp = psX.tile([12, 256], f32, name="gtp")
                nc.tensor.matmul(gtp[:], atT[:, 0, :], wa2[:, 0, :],
                                 start=True, stop=False)
                nc.tensor.matmul(gtp[:], atT[:, 1, :], wa2[:, 1, :],
                                 start=False, stop=True)
                GT = const.tile([12, 256], f32)
                nc.vector.tensor_copy(GT[:], gtp[:])

            # ---- phase D: final = G @ q + b + x ----
            with ExitStack() as dctx:
                psO = dctx.enter_context(tc.tile_pool(name="psO", bufs=2, space="PSUM"))
                sbD = dctx.enter_context(tc.tile_pool(name="sbD", bufs=3))
                xpoolD = dctx.enter_context(tc.tile_pool(name="xpoolD", bufs=2))
                qpool = dctx.enter_context(tc.tile_pool(name="qpool", bufs=2))
                for j in range(16):
                    qf = qpool.tile([12, 4, W], f32, name="qf")
                    nc.sync.dma_start(qf[:], dgd[:, 4 * j:4 * j + 4, :])
                    xr4 = xpoolD.tile([P, 2, 4, W], f32, name="xr4")
                    nc.sync.dma_start(xr4[:], xin[:, :, 4 * j:4 * j + 4, :])
                    for m in range(2):
                        op_ = psO.tile([P, 4, W], f32, name="op_")
                        nc.tensor.matmul(op_[:], GT[:, 128 * m:128 * (m + 1)],
                                         qf[:], start=True, stop=True)
                        ts_ = sbD.tile([P, 4, W], f32, name="ts_")
                        nc.scalar.activation(ts_[:], op_[:], AF.Identity,
                                             bias=b2a[:, m:m + 1])
                        nc.vector.tensor_tensor(ts_[:], ts_[:], xr4[:, m, :, :],
                                                AluOpType.add)
                        nc.sync.dma_start(fin_o[:, m, 4 * j:4 * j + 4, :], ts_[:])

    nc.compile()
    return nc


def _prep_shared(inputs):
    """Host-side weight folding; returns dict of per-flip weight tensors."""
    f = np.float32
    g1 = np.asarray(inputs['g1'], f); v1 = np.asarray(inputs['v1'], f)
    m1 = np.asarray(inputs['m1'], f); be1 = np.asarray(inputs['be1'], f)
    s1 = g1 / np.sqrt(v1 + np.float32(BN_EPS))
    w_do1 = np.asarray(inputs['w_do1'], f)
    b1 = ((np.asarray(inputs['b_do1'], f) - m1) * s1 + be1).astype(f)
    g2 = np.asarray(inputs['g2'], f); v2 = np.asarray(inputs['v2'], f)
    m2 = np.asarray(inputs['m2'], f); be2 = np.asarray(inputs['be2'], f)
    s2 = g2 / np.sqrt(v2 + np.float32(BN_EPS))
    wa1 = (np.asarray(inputs['w_acf1'], f)[:, :, 0, 0] * s2[:, None]).astype(f)
    ba1 = (be2 - m2 * s2).astype(f)
    wa2 = np.asarray(inputs['w_acf2'], f)[:, :, 0, 0]
    ba2 = np.asarray(inputs['b_acf2'], f)
    w2 = np.asarray(inputs['w_do2'], f)[:, :, 0, 0]
    b2 = np.asarray(inputs['b_do2'], f)
    wdd = np.asarray(inputs['w_dd'], f)

    _, _, Ae, Be, Ao, Bo = _tables()
    colw = np.concatenate([Ae, Be, Ao, Bo]).astype(f)[None, :]

    shared = {
        'w2t': np.ascontiguousarray(w2.T),                      # (128, 96)
        'b2d': b2[:, None].copy(),
        'bddd': np.asarray(inputs['b_dd'], f)[:, None].copy(),
        'wa1t': np.ascontiguousarray(
            wa1.T.reshape(2, 128, 256).transpose(1, 0, 2)),     # (128,2,256)
        'b2row': ba1[None, :].copy(),
        'wa2t': np.ascontiguousarray(
            wa2.T.reshape(2, 128, 256).transpose(1, 0, 2)),     # (128,2,256)
        'b2at': np.ascontiguousarray(ba2.reshape(2, 128).T),    # (128, 2)
        'colw': colw,
        'identd': np.eye(16, dtype=f),
    }
    per_flip = []
    for flip in (False, True):
        w1 = w_do1 * s1[:, None, None, None]
        if flip:
            w1 = w1[:, :, ::-1, :]
        # (128co, 256ci, 3, 3) -> w1t[ci_sub, g, 3dy+dx, co]
        w1t = np.zeros((128, 2, 9, 128), f)
        for dy in range(3):
            for dx in range(3):
                wt = w1[:, :, dy, dx].T.astype(f)               # (256ci, 128co)
                w1t[:, 0, 3 * dy + dx, :] = wt[:128]
                w1t[:, 1, 3 * dy + dx, :] = wt[128:]
        wddf = wdd[:, :, ::-1, :] if flip else wdd
        wddt = np.zeros((96, 9, 12), f)
        for co in range(12):
            for jj in range(8):
                for dy in range(3):
                    for dx in range(3):
                        wddt[co * 8 + jj, 3 * dy + dx, co] = 0.25 * wddf[co, jj, dy, dx]
        per_flip.append({'w1t': w1t, 'wddt': wddt, 'b1d': b1[:, None].copy()})
    return shared, per_flip


def kernel(**inputs):
    from concourse.bass_utils import run_bass_kernel_spmd

    key = ('nc', USE_F32R)
    if key not in _cache:
        _cache[key] = _build_nc(USE_F32R)
    nc = _cache[key]

    x = np.asarray(inputs['x'], np.float32)
    N = x.shape[0]
    shared, per_flip = _prep_shared(inputs)
    in_maps = []
    for c in range(N_CORES):
        n, flip = c // 2, c % 2
        xf = x[n] if not flip else x[n][:, ::-1, :]
        xin = np.ascontiguousarray(
            xf[:, :XR, :].reshape(2, 128, XR, W).transpose(1, 0, 2, 3))
        m = {'xin': xin}
        m.update(shared)
        m.update(per_flip[flip])
        in_maps.append(m)

    res = run_bass_kernel_spmd(nc, in_maps, list(range(N_CORES)))

    depth = np.zeros((N, 96, 256, 256), np.float32)
    dguide = np.zeros((N, 12, 128, 128), np.float32)
    final = np.zeros((N, 256, 128, 128), np.float32)
    for c in range(N_CORES):
        n, flip = c // 2, c % 2
        r = res.results[c]
        dep = r['depth_o']
        dg = r['dg_o']
        fin = r['fin_o'].transpose(1, 0, 2, 3).reshape(256, 64, W)
        if not flip:
            depth[n, :, :128, :] = dep
            dguide[n, :, :64, :] = dg
            final[n, :, :64, :] = fin
        else:
            depth[n, :, 128:, :] = dep[:, ::-1, :]
            dguide[n, :, 64:, :] = dg[:, ::-1, :]
            final[n, :, 64:, :] = fin[:, ::-1, :]
    return depth, dguide, final
